# revision 10
# baseline (speedup 1.0000x reference)
"""Multi-head cross-attention (MHAForCrossFusion) on 8 Trainium2 cores.

Sharding: core = (batch, head-group). Core 4*b+j owns batch b and heads
4j..4j+3 (CW=256 projection features). Each core reads only its batch's
q/k/v (host pre-transposed to feature-major, cast to bf16) and writes a
full-width fp32 partial of its batch's output rows; host sums 4 partials
per batch + bo.

Per-core device program (matmul operands bf16, fp32 PSUM accumulate):
 - K/Q projections feature-major [feat, tok]; V projected token-major
   (activation tile as the stationary operand) straight into the
   ones-augmented vma layout [key, head*(hv|1)] (softmax denom trick)
 - attention per (head-pair g, 512-query chunk): scores S.T = km.T @ qm
   per head over each 128-key tile; the two heads of the pair are
   row-packed in the PE array (tile_position) and land in adjacent PSUM
   banks, so exp runs as one ACT instruction over [128, 2*512]
 - ctx_aug[0:65] = [vm | 1].T @ expS accumulated over key tiles;
   row 64 = softmax denominator
 - normalize: reciprocal of the denom row, K=1 matmul broadcast across
   partitions, DVE multiply -> ctxn
 - out-projection: out[t, :] = ctxn.T @ Wo_slice.T (partial sum)
"""

import os

import numpy as np
import ml_dtypes

import concourse.bass as bass
import concourse.mybir as mybir
import concourse.tile as tile
from concourse import bass_utils

N_CORES = 8
B, L, D = 2, 2048, 1024
NH, HD = 16, 64
HG = NH // (N_CORES // B)  # 4 heads per core
CW = HG * HD  # 256 projection features per core
SCALE = 1.0 / np.sqrt(HD)

F32 = mybir.dt.float32
BF16 = mybir.dt.bfloat16

DC = D // 128  # 8 contraction tiles for the projections
NT = L // 128  # 16 key tiles
NCH = L // 512  # 4 token chunks


def _split_matmul_waits(nc):
    """fp32/fp32r matmuls lower to a self-loading LDW whose ISA struct has a
    single sem-wait slot (HWDGE DMA likewise); walrus rejects >1 wait. Move
    extra waits onto same-engine NoOps inserted right before the matmul
    (program order on the sequencer preserves the happens-before)."""
    for f in nc.m.functions:
        for bb in f.blocks:
            insts = list(bb.instructions)
            out = []
            for inst in insts:
                si = inst.sync_info
                if si is not None and len(si.on_wait) > 1:
                    for w in si.on_wait[:-1]:
                        nop = mybir.InstNoOp(
                            name=nc.get_next_instruction_name(),
                            ins=[],
                            outs=[],
                            engine=inst.engine,
                            bass_nofuse=True,
                        )
                        nop.sync_info = mybir.SyncInfo(on_wait=[w], on_update=[])
                        out.append(nop)
                    inst.sync_info = mybir.SyncInfo(
                        on_wait=[si.on_wait[-1]], on_update=si.on_update
                    )
                out.append(inst)
            if len(out) != len(insts):
                bb.instructions = out
    return nc


def build_nc():
    nc = bass.Bass("TRN2", target_bir_lowering=False, debug=False)

    qT = nc.dram_tensor("qT", [D, L], BF16, kind="ExternalInput").ap()
    kT = nc.dram_tensor("kT", [D, L], BF16, kind="ExternalInput").ap()
    vT = nc.dram_tensor("vT", [D, L], BF16, kind="ExternalInput").ap()
    wqt = nc.dram_tensor("wqt", [D, CW], BF16, kind="ExternalInput").ap()
    wkt = nc.dram_tensor("wkt", [D, CW], BF16, kind="ExternalInput").ap()
    wvt = nc.dram_tensor("wvt", [D, CW], BF16, kind="ExternalInput").ap()
    wot = nc.dram_tensor("wot", [CW, D], BF16, kind="ExternalInput").ap()
    bq = nc.dram_tensor("bq", [CW, 1], F32, kind="ExternalInput").ap()
    bk = nc.dram_tensor("bk", [CW, 1], F32, kind="ExternalInput").ap()
    bvb = nc.dram_tensor("bvb", [128, CW], F32, kind="ExternalInput").ap()
    out_p = nc.dram_tensor("out_p", [L, D], F32, kind="ExternalOutput").ap()

    with tile.TileContext(nc) as tc:
        with (
            tc.tile_pool(name="singles", bufs=1) as singles,
            tc.tile_pool(name="acts", bufs=1) as acts,
            tc.tile_pool(name="stage", bufs=3) as stage,
            tc.tile_pool(name="small", bufs=3) as small,
            tc.tile_pool(name="psq", bufs=2, space="PSUM") as ppq,
            tc.tile_pool(name="psc", bufs=4, space="PSUM") as ppc,
        ):
            ones = singles.tile([1, 64], F32)
            nc.vector.memset(ones, 1.0)

            w_sb = {}
            for name, dram in (("wq", wqt), ("wk", wkt), ("wv", wvt)):
                w = singles.tile([128, DC, CW], BF16, name=name + "_sb")
                nc.sync.dma_start(w, dram.rearrange("(c p) h -> p c h", p=128))
                w_sb[name] = w
            wot_sb = singles.tile([128, 2, D], BF16)
            nc.sync.dma_start(wot_sb, wot.rearrange("(g p) d -> p g d", p=128))
            b_sb = {}
            for name, dram in (("bq", bq), ("bk", bk)):
                b = singles.tile([128, 2], F32, name=name + "_sb")
                nc.sync.dma_start(b, dram.rearrange("(g p) one -> p (g one)", p=128))
                b_sb[name] = b
            bvb_sb = singles.tile([128, CW], F32)
            nc.sync.dma_start(bvb_sb, bvb)

            qm = acts.tile([128, 2, L], BF16)  # feature-major projections
            km = acts.tile([128, 2, L], BF16)
            vma = acts.tile([128, NT, HG * 66], BF16)  # [key, kt, (hv|1) x4 heads]
            ctxn = acts.tile([128, 2, L], BF16)

            # ones columns of the augmented V (col 64 of each head's 66)
            nc.vector.memset(
                vma.rearrange("p t (h c) -> p t h c", c=66)[:, :, :, 64], 1.0
            )

            # ---- phase 1: K/V then Q projections, per 512-token chunk ----
            for name, dram in (("k", kT), ("v", vT), ("q", qT)):
                for ci in range(NCH):
                    ts = slice(ci * 512, (ci + 1) * 512)
                    xc = stage.tile([128, DC, 512], BF16, tag="xc", name=f"{name}c")
                    nc.sync.dma_start(
                        xc, dram.rearrange("(c p) t -> p c t", p=128)[:, :, ts]
                    )
                    if name == "v":
                        # token-major: vm[t, f], activations as the
                        # stationary operand
                        for tt in range(4):
                            ps = ppq.tile([128, 2, 512], F32, tag="sq")
                            for dc in range(DC):
                                nc.tensor.matmul(
                                    ps[:, 0, 0:CW],
                                    lhsT=xc[:, dc, tt * 128 : (tt + 1) * 128],
                                    rhs=w_sb["wv"][:, dc, :],
                                    start=(dc == 0),
                                    stop=(dc == DC - 1),
                                )
                            nc.vector.tensor_add(
                                vma.rearrange("p t (h c) -> p t h c", c=66)[
                                    :, ci * 4 + tt, :, 0:64
                                ],
                                ps[:, 0, :].rearrange("p (h c) -> p h c", c=64)[
                                    :, 0:HG, :
                                ],
                                bvb_sb.rearrange("p (h c) -> p h c", c=64),
                            )
                    else:
                        dst = km if name == "k" else qm
                        bias = b_sb["bk" if name == "k" else "bq"]
                        for g in range(2):
                            ps = ppq.tile([128, 2, 512], F32, tag="sq")
                            for dc in range(DC):
                                nc.tensor.matmul(
                                    ps[:, 0, :],
                                    lhsT=w_sb["w" + name][
                                        :, dc, g * 128 : (g + 1) * 128
                                    ],
                                    rhs=xc[:, dc, :],
                                    start=(dc == 0),
                                    stop=(dc == DC - 1),
                                )
                            nc.vector.tensor_scalar_add(
                                dst[:, g, ts], ps[:, 0, :], bias[:, g : g + 1]
                            )

            # ---- phase 2: attention per (head pair g, 512-query chunk) ----
            # The normalize of chunk n is deferred into the middle of chunk
            # n+1's key loop: the PE executes matmuls in order, so a
            # broadcast matmul emitted right after the reciprocal would
            # stall the whole PE stream on the DVE reciprocal latency.
            def emit_normalize(pg, pls, pctx, prcs):
                for h in range(2):
                    nc.tensor.matmul(pctx[h][64:128, :], lhsT=ones, rhs=prcs[h])
                    bcs = small.tile([64, 512], F32, tag="bcs")
                    nc.vector.tensor_copy(bcs, pctx[h][64:128, :])
                    nc.vector.tensor_mul(
                        ctxn[h * 64 : (h + 1) * 64, pg, pls], pctx[h][0:64, :], bcs
                    )

            pending = None
            for g in range(2):
                for lc in range(NCH):
                    ls = slice(lc * 512, (lc + 1) * 512)
                    ctx = [
                        ppc.tile([128, 512], F32, tag="ctx", name=f"ctx{h}")
                        for h in range(2)
                    ]
                    for pt in range(NT):
                        ks = slice(pt * 128, (pt + 1) * 128)
                        squad = ppq.tile([128, 2, 512], F32, tag="sq")
                        for h in range(2):
                            hs = slice(h * 64, (h + 1) * 64)
                            nc.tensor.matmul(
                                squad[:, h, :],
                                lhsT=km[hs, g, ks],
                                rhs=qm[hs, g, ls],
                                tile_position=(h * 64, 0),
                            )
                        es = small.tile([128, 2, 512], BF16, tag="es")
                        nc.scalar.activation(
                            es, squad, mybir.ActivationFunctionType.Exp, scale=SCALE
                        )
                        for h in range(2):
                            nc.tensor.matmul(
                                ctx[h][0:65, :],
                                lhsT=vma[
                                    :, pt, (g * 2 + h) * 66 : (g * 2 + h) * 66 + 65
                                ],
                                rhs=es[:, h, :],
                                start=(pt == 0),
                                stop=(pt == NT - 1),
                            )
                        if pt == 7 and pending is not None:
                            emit_normalize(*pending)
                            pending = None
                    rcs = []
                    for h in range(2):
                        rc = small.tile([1, 512], F32, tag="rc")
                        nc.vector.reciprocal(rc, ctx[h][64:65, :])
                        rcs.append(rc)
                    pending = (g, ls, ctx, rcs)
            emit_normalize(*pending)

            # ---- phase 3: out-projection, per 128-token tile ----
            for lc in range(NCH):
                for tt in range(4):
                    t0 = lc * 512 + tt * 128
                    ob = small.tile([128, D], F32, tag="ob")
                    for half in range(2):
                        po = ppq.tile([128, 2, 512], F32, tag="sq")
                        for g in range(2):
                            nc.tensor.matmul(
                                po[:, 0, :],
                                lhsT=ctxn[:, g, t0 : t0 + 128],
                                rhs=wot_sb[:, g, half * 512 : (half + 1) * 512],
                                start=(g == 0),
                                stop=(g == 1),
                            )
                        nc.vector.tensor_copy(
                            ob[:, half * 512 : (half + 1) * 512], po[:, 0, :]
                        )
                    nc.sync.dma_start(out_p[t0 : t0 + 128, :], ob)
    return _split_matmul_waits(nc)


_NC_CACHE = None


def kernel(q, k, v, attention_mask, Wq, bq, Wk, bk, Wv, bv, Wo, bo):
    global _NC_CACHE
    q, k, v = (np.asarray(x, np.float32) for x in (q, k, v))
    assert np.asarray(attention_mask).all(), "kernel assumes all-ones mask"
    if _NC_CACHE is None:
        _NC_CACHE = build_nc()
    nc = _NC_CACHE

    bfc = lambda x: np.ascontiguousarray(np.asarray(x, ml_dtypes.bfloat16))
    c = np.ascontiguousarray
    Wq, Wk, Wv, Wo = (np.asarray(x, np.float32) for x in (Wq, Wk, Wv, Wo))
    bq, bk, bv, bo = (np.asarray(x, np.float32) for x in (bq, bk, bv, bo))

    qT = [bfc(q[b].T) for b in range(B)]
    kT = [bfc(k[b].T) for b in range(B)]
    vT = [bfc(v[b].T) for b in range(B)]

    in_maps = []
    for ci in range(N_CORES):
        b = ci // (N_CORES // B)
        j = ci % (N_CORES // B)
        hs = slice(j * CW, (j + 1) * CW)
        in_maps.append(
            {
                "qT": qT[b],
                "kT": kT[b],
                "vT": vT[b],
                "wqt": bfc(Wq.T[:, hs]),
                "wkt": bfc(Wk.T[:, hs]),
                "wvt": bfc(Wv.T[:, hs]),
                "wot": bfc(Wo.T[hs, :]),
                "bq": c(bq[hs, None]),
                "bk": c(bk[hs, None]),
                "bvb": c(np.broadcast_to(bv[hs][None, :], (128, CW))),
            }
        )

    res = bass_utils.run_bass_kernel_spmd(
        nc,
        in_maps,
        core_ids=list(range(N_CORES)),
        tmpdir=os.environ.get("KERNEL_TMPDIR"),
    )
    globals()["LAST_RES"] = res
    out = np.zeros((B, L, D), np.float32)
    for ci, r in enumerate(res.results):
        out[ci // (N_CORES // B)] += r["out_p"]
    out += bo[None, None, :]
    return out


# revision 13
# speedup vs baseline: 1.0546x; 1.0546x over previous
"""Multi-head cross-attention (MHAForCrossFusion) on 8 Trainium2 cores.

Sharding: core = (batch, head-group). Core 4*b+j owns batch b and heads
4j..4j+3 (CW=256 projection features). Each core reads only its batch's
q/k/v (host pre-transposed to feature-major, cast to bf16) and writes a
full-width fp32 partial of its batch's output rows; host sums 4 partials
per batch + bo.

Per-core device program (matmul operands bf16, fp32 PSUM accumulate):
 - all input DMAs issued up-front; K/Q projections feature-major
   [feat, tok]; V projected token-major (activation tile as the
   stationary operand) straight into the ones-augmented vma layout
   [key, head*(hv|1)] (softmax denominator trick)
 - attention per (512-query chunk lc, head-pair g): scores S.T = km.T @
   qm per head over each 128-key tile; the two heads of the pair are
   row-packed in the PE array (tile_position) and land in adjacent PSUM
   banks, so exp runs as one ACT instruction over [128, 2*512]
 - ctx_aug[0:65] = [vm | 1].T @ expS accumulated over key tiles;
   row 64 = softmax denominator
 - attention starts after only the first k/v/q chunks are projected;
   remaining projections, normalizes and out-projections are deferred
   "jobs" drained inside later key loops so ACT (the bottleneck engine)
   never waits on a cold pipeline
 - normalize: ctx evacuated to SBUF (releases the PSUM bank), DVE
   reciprocal of the denom row, K=1 matmul broadcast across partitions,
   DVE multiply -> ctxn
 - out-projection: out[t, :] = ctxn.T @ Wo_slice.T (partial sum)
"""

import os
from collections import deque

import numpy as np
import ml_dtypes

import concourse.bass as bass
import concourse.mybir as mybir
import concourse.tile as tile
from concourse import bass_utils

N_CORES = 8
B, L, D = 2, 2048, 1024
NH, HD = 16, 64
HG = NH // (N_CORES // B)  # 4 heads per core
CW = HG * HD  # 256 projection features per core
SCALE = 1.0 / np.sqrt(HD)

F32 = mybir.dt.float32
BF16 = mybir.dt.bfloat16

DC = D // 128  # 8 contraction tiles for the projections
NT = L // 128  # 16 key tiles
NCH = L // 512  # 4 token chunks


def _split_matmul_waits(nc):
    """fp32/fp32r matmuls lower to a self-loading LDW whose ISA struct has a
    single sem-wait slot (HWDGE DMA likewise); walrus rejects >1 wait. Move
    extra waits onto same-engine NoOps inserted right before the matmul
    (program order on the sequencer preserves the happens-before)."""
    for f in nc.m.functions:
        for bb in f.blocks:
            insts = list(bb.instructions)
            out = []
            for inst in insts:
                si = inst.sync_info
                if si is not None and len(si.on_wait) > 1:
                    for w in si.on_wait[:-1]:
                        nop = mybir.InstNoOp(
                            name=nc.get_next_instruction_name(),
                            ins=[],
                            outs=[],
                            engine=inst.engine,
                            bass_nofuse=True,
                        )
                        nop.sync_info = mybir.SyncInfo(on_wait=[w], on_update=[])
                        out.append(nop)
                    inst.sync_info = mybir.SyncInfo(
                        on_wait=[si.on_wait[-1]], on_update=si.on_update
                    )
                out.append(inst)
            if len(out) != len(insts):
                bb.instructions = out
    return nc


def build_nc():
    nc = bass.Bass("TRN2", target_bir_lowering=False, debug=False)

    qT = nc.dram_tensor("qT", [D, L], BF16, kind="ExternalInput").ap()
    kT = nc.dram_tensor("kT", [D, L], BF16, kind="ExternalInput").ap()
    vT = nc.dram_tensor("vT", [D, L], BF16, kind="ExternalInput").ap()
    wqt = nc.dram_tensor("wqt", [D, CW], BF16, kind="ExternalInput").ap()
    wkt = nc.dram_tensor("wkt", [D, CW], BF16, kind="ExternalInput").ap()
    wvt = nc.dram_tensor("wvt", [D, CW], BF16, kind="ExternalInput").ap()
    wot = nc.dram_tensor("wot", [CW, D], BF16, kind="ExternalInput").ap()
    bq = nc.dram_tensor("bq", [CW, 1], F32, kind="ExternalInput").ap()
    bk = nc.dram_tensor("bk", [CW, 1], F32, kind="ExternalInput").ap()
    bvb = nc.dram_tensor("bvb", [128, CW], F32, kind="ExternalInput").ap()
    out_p = nc.dram_tensor("out_p", [L, D], F32, kind="ExternalOutput").ap()

    with tile.TileContext(nc) as tc:
        with (
            tc.tile_pool(name="singles", bufs=1) as singles,
            tc.tile_pool(name="acts", bufs=1) as acts,
            tc.tile_pool(name="stage", bufs=12) as stage,
            tc.tile_pool(name="small", bufs=3) as small,
            tc.tile_pool(name="norm", bufs=5) as normp,
            tc.tile_pool(name="psq", bufs=3, space="PSUM") as ppq,
            tc.tile_pool(name="psc", bufs=2, space="PSUM") as ppc,
        ):
            ones = singles.tile([1, 64], F32)
            nc.vector.memset(ones, 1.0)

            w_sb = {}
            for name, dram in (("wq", wqt), ("wk", wkt), ("wv", wvt)):
                w = singles.tile([128, DC, CW], BF16, name=name + "_sb")
                nc.sync.dma_start(w, dram.rearrange("(c p) h -> p c h", p=128))
                w_sb[name] = w
            wot_sb = singles.tile([128, 2, D], BF16)
            nc.sync.dma_start(wot_sb, wot.rearrange("(g p) d -> p g d", p=128))
            b_sb = {}
            for name, dram in (("bq", bq), ("bk", bk)):
                b = singles.tile([128, 2], F32, name=name + "_sb")
                nc.sync.dma_start(b, dram.rearrange("(g p) one -> p (g one)", p=128))
                b_sb[name] = b
            bvb_sb = singles.tile([128, CW], F32)
            nc.sync.dma_start(bvb_sb, bvb)

            # all q/k/v chunk DMAs up-front: the DMA engines run far ahead
            # of compute, so every projection finds its data resident
            xcs = {}
            for name, dram in (("k", kT), ("v", vT), ("q", qT)):
                for ci in range(NCH):
                    xc = stage.tile(
                        [128, DC, 512], BF16, tag="xc", name=f"{name}c{ci}"
                    )
                    nc.sync.dma_start(
                        xc,
                        dram.rearrange("(c p) t -> p c t", p=128)[
                            :, :, ci * 512 : (ci + 1) * 512
                        ],
                    )
                    xcs[name, ci] = xc

            qm = acts.tile([128, 2, L], BF16)  # feature-major projections
            km = acts.tile([128, 2, L], BF16)
            vma = acts.tile([128, NT, HG * 66], BF16)  # [key, kt, (hv|1) x4]
            ctxn = acts.tile([128, 2, L], BF16)

            # ones columns of the augmented V (col 64 of each head's 66)
            nc.vector.memset(
                vma.rearrange("p t (h c) -> p t h c", c=66)[:, :, :, 64], 1.0
            )

            # warm the ACT exp table set during phase 1 instead of paying
            # the ~2.7us PSEUDO_LOAD at the first real exp
            dummy = singles.tile([128, 2], F32)
            nc.vector.memset(dummy[:, 0:1], 0.0)
            nc.scalar.activation(
                dummy[:, 1:2], dummy[:, 0:1], mybir.ActivationFunctionType.Exp
            )

            def fproj(name, ci):
                # feature-major projection of one 512-token chunk (k or q)
                dst, bias = (km, b_sb["bk"]) if name == "k" else (qm, b_sb["bq"])
                ts = slice(ci * 512, (ci + 1) * 512)
                for g in range(2):
                    ps = ppq.tile([128, 2, 512], F32, tag="sq")
                    for dc in range(DC):
                        nc.tensor.matmul(
                            ps[:, 0, :],
                            lhsT=w_sb["w" + name][:, dc, g * 128 : (g + 1) * 128],
                            rhs=xcs[name, ci][:, dc, :],
                            start=(dc == 0),
                            stop=(dc == DC - 1),
                        )
                    nc.vector.tensor_scalar_add(
                        dst[:, g, ts], ps[:, 0, :], bias[:, g : g + 1]
                    )

            def vproj(ci):
                # token-major projection: vm[t, f], activations stationary
                for tt in range(4):
                    ps = ppq.tile([128, 2, 512], F32, tag="sq")
                    for dc in range(DC):
                        nc.tensor.matmul(
                            ps[:, 0, 0:CW],
                            lhsT=xcs["v", ci][:, dc, tt * 128 : (tt + 1) * 128],
                            rhs=w_sb["wv"][:, dc, :],
                            start=(dc == 0),
                            stop=(dc == DC - 1),
                        )
                    nc.vector.tensor_add(
                        vma.rearrange("p t (h c) -> p t h c", c=66)[
                            :, ci * 4 + tt, :, 0:64
                        ],
                        ps[:, 0, :].rearrange("p (h c) -> p h c", c=64)[:, 0:HG, :],
                        bvb_sb.rearrange("p (h c) -> p h c", c=64),
                    )

            def emit_normalize(pg, pls, cus, prcs):
                for h in range(2):
                    po = ppq.tile([128, 2, 512], F32, tag="sq")
                    nc.tensor.matmul(po[0:64, 0, :], lhsT=ones, rhs=prcs[h])
                    nc.vector.tensor_mul(
                        ctxn[h * 64 : (h + 1) * 64, pg, pls],
                        cus[h][0:64, :],
                        po[0:64, 0, :],
                    )

            def emit_outproj(lc):
                for tt in range(4):
                    t0 = lc * 512 + tt * 128
                    ob = small.tile([128, D], F32, tag="ob")
                    for half in range(2):
                        po = ppq.tile([128, 2, 512], F32, tag="sq")
                        for g in range(2):
                            nc.tensor.matmul(
                                po[:, 0, :],
                                lhsT=ctxn[:, g, t0 : t0 + 128],
                                rhs=wot_sb[:, g, half * 512 : (half + 1) * 512],
                                start=(g == 0),
                                stop=(g == 1),
                            )
                        nc.vector.tensor_copy(
                            ob[:, half * 512 : (half + 1) * 512], po[:, 0, :]
                        )
                    nc.sync.dma_start(out_p[t0 : t0 + 128, :], ob)

            # phase 1 head: just enough to start attention
            fproj("k", 0)
            vproj(0)
            fproj("q", 0)
            jobs = deque()
            for ci in range(1, NCH):
                jobs.append(lambda ci=ci: fproj("k", ci))
                jobs.append(lambda ci=ci: vproj(ci))

            # ---- phase 2: attention; deferred jobs drain at odd key tiles
            for lc in range(NCH):
                ls = slice(lc * 512, (lc + 1) * 512)
                for g in range(2):
                    ctx = [
                        ppc.tile([128, 512], F32, tag="ctx", name=f"ctx{h}")
                        for h in range(2)
                    ]
                    for pt in range(NT):
                        ks = slice(pt * 128, (pt + 1) * 128)
                        squad = ppq.tile([128, 2, 512], F32, tag="sq")
                        for h in range(2):
                            hs = slice(h * 64, (h + 1) * 64)
                            nc.tensor.matmul(
                                squad[:, h, :],
                                lhsT=km[hs, g, ks],
                                rhs=qm[hs, g, ls],
                                tile_position=(h * 64, 0),
                            )
                        es = small.tile([128, 2, 512], BF16, tag="es")
                        nc.scalar.activation(
                            es, squad, mybir.ActivationFunctionType.Exp, scale=SCALE
                        )
                        for h in range(2):
                            nc.tensor.matmul(
                                ctx[h][0:65, :],
                                lhsT=vma[
                                    :, pt, (g * 2 + h) * 66 : (g * 2 + h) * 66 + 65
                                ],
                                rhs=es[:, h, :],
                                start=(pt == 0),
                                stop=(pt == NT - 1),
                            )
                        if pt % 2 == 1 and jobs:
                            jobs.popleft()()
                    # evacuate ctx to SBUF (frees the PSUM banks) and start
                    # the reciprocal; the normalize runs as a later job
                    cus, rcs = [], []
                    for h in range(2):
                        cu = normp.tile([65, 512], F32, tag="cu")
                        nc.vector.tensor_copy(cu, ctx[h][0:65, :])
                        rc = normp.tile([1, 512], F32, tag="rc")
                        nc.vector.reciprocal(rc, cu[64:65, :])
                        cus.append(cu)
                        rcs.append(rc)
                    if g == 0 and lc < NCH - 1:
                        jobs.append(lambda ci=lc + 1: fproj("q", ci))
                    jobs.append(
                        lambda g=g, ls=ls, cus=cus, rcs=rcs: emit_normalize(
                            g, ls, cus, rcs
                        )
                    )
                    if g == 1:
                        jobs.append(lambda lc=lc: emit_outproj(lc))
            while jobs:
                jobs.popleft()()
    return _split_matmul_waits(nc)


_NC_CACHE = None


def kernel(q, k, v, attention_mask, Wq, bq, Wk, bk, Wv, bv, Wo, bo):
    global _NC_CACHE
    q, k, v = (np.asarray(x, np.float32) for x in (q, k, v))
    assert np.asarray(attention_mask).all(), "kernel assumes all-ones mask"
    if _NC_CACHE is None:
        _NC_CACHE = build_nc()
    nc = _NC_CACHE

    bfc = lambda x: np.ascontiguousarray(np.asarray(x, ml_dtypes.bfloat16))
    c = np.ascontiguousarray
    Wq, Wk, Wv, Wo = (np.asarray(x, np.float32) for x in (Wq, Wk, Wv, Wo))
    bq, bk, bv, bo = (np.asarray(x, np.float32) for x in (bq, bk, bv, bo))

    qT = [bfc(q[b].T) for b in range(B)]
    kT = [bfc(k[b].T) for b in range(B)]
    vT = [bfc(v[b].T) for b in range(B)]

    in_maps = []
    for ci in range(N_CORES):
        b = ci // (N_CORES // B)
        j = ci % (N_CORES // B)
        hs = slice(j * CW, (j + 1) * CW)
        in_maps.append(
            {
                "qT": qT[b],
                "kT": kT[b],
                "vT": vT[b],
                "wqt": bfc(Wq.T[:, hs]),
                "wkt": bfc(Wk.T[:, hs]),
                "wvt": bfc(Wv.T[:, hs]),
                "wot": bfc(Wo.T[hs, :]),
                "bq": c(bq[hs, None]),
                "bk": c(bk[hs, None]),
                "bvb": c(np.broadcast_to(bv[hs][None, :], (128, CW))),
            }
        )

    res = bass_utils.run_bass_kernel_spmd(
        nc,
        in_maps,
        core_ids=list(range(N_CORES)),
        tmpdir=os.environ.get("KERNEL_TMPDIR"),
    )
    globals()["LAST_RES"] = res
    out = np.zeros((B, L, D), np.float32)
    for ci, r in enumerate(res.results):
        out[ci // (N_CORES // B)] += r["out_p"]
    out += bo[None, None, :]
    return out


# revision 14
# speedup vs baseline: 1.1289x; 1.0705x over previous
"""Multi-head cross-attention (MHAForCrossFusion) on 8 Trainium2 cores.

Sharding: core = (batch, head-group). Core 4*b+j owns batch b and heads
4j..4j+3 (CW=256 projection features). Each core reads only its batch's
q/k/v (host pre-transposed to feature-major, cast to bf16) and writes a
full-width fp32 partial of its batch's output rows; host sums 4 partials
per batch + bo.

Per-core device program (matmul operands bf16, fp32 PSUM accumulate):
 - all input DMAs issued up-front; K/Q projections feature-major
   [feat, tok]; V projected token-major (activation tile as the
   stationary operand) straight into the ones-augmented vma layout
   [key, head*(hv|1)] (softmax denominator trick)
 - attention per (512-query chunk lc, head-pair g): scores S.T = km.T @
   qm per head over each 128-key tile; the two heads of the pair are
   row-packed in the PE array (tile_position) and land in adjacent PSUM
   banks, so exp runs as one ACT instruction over [128, 2*512]
 - ctx_aug[0:65] = [vm | 1].T @ expS accumulated over key tiles;
   row 64 = softmax denominator
 - attention starts after only the first k/v/q chunks are projected;
   remaining projections, normalizes and out-projections are deferred
   "jobs" drained inside later key loops so ACT (the bottleneck engine)
   never waits on a cold pipeline
 - normalize: ctx evacuated to SBUF (releases the PSUM bank), DVE
   reciprocal of the denom row, K=1 matmul broadcast across partitions,
   DVE multiply -> ctxn
 - out-projection: out[t, :] = ctxn.T @ Wo_slice.T (partial sum)
"""

import os
from collections import deque

import numpy as np
import ml_dtypes

import concourse.bass as bass
import concourse.mybir as mybir
import concourse.tile as tile
from concourse import bass_utils

N_CORES = 8
B, L, D = 2, 2048, 1024
NH, HD = 16, 64
HG = NH // (N_CORES // B)  # 4 heads per core
CW = HG * HD  # 256 projection features per core
SCALE = 1.0 / np.sqrt(HD)

F32 = mybir.dt.float32
BF16 = mybir.dt.bfloat16

DC = D // 128  # 8 contraction tiles for the projections
NT = L // 128  # 16 key tiles
NCH = L // 512  # 4 token chunks


def _split_matmul_waits(nc):
    """fp32/fp32r matmuls lower to a self-loading LDW whose ISA struct has a
    single sem-wait slot (HWDGE DMA likewise); walrus rejects >1 wait. Move
    extra waits onto same-engine NoOps inserted right before the matmul
    (program order on the sequencer preserves the happens-before)."""
    for f in nc.m.functions:
        for bb in f.blocks:
            insts = list(bb.instructions)
            out = []
            for inst in insts:
                si = inst.sync_info
                if si is not None and len(si.on_wait) > 1:
                    for w in si.on_wait[:-1]:
                        nop = mybir.InstNoOp(
                            name=nc.get_next_instruction_name(),
                            ins=[],
                            outs=[],
                            engine=inst.engine,
                            bass_nofuse=True,
                        )
                        nop.sync_info = mybir.SyncInfo(on_wait=[w], on_update=[])
                        out.append(nop)
                    inst.sync_info = mybir.SyncInfo(
                        on_wait=[si.on_wait[-1]], on_update=si.on_update
                    )
                out.append(inst)
            if len(out) != len(insts):
                bb.instructions = out
    return nc


def build_nc():
    nc = bass.Bass("TRN2", target_bir_lowering=False, debug=False)

    qT = nc.dram_tensor("qT", [D, L], BF16, kind="ExternalInput").ap()
    kT = nc.dram_tensor("kT", [D, L], BF16, kind="ExternalInput").ap()
    vT = nc.dram_tensor("vT", [D, L], BF16, kind="ExternalInput").ap()
    wqt = nc.dram_tensor("wqt", [D, CW], BF16, kind="ExternalInput").ap()
    wkt = nc.dram_tensor("wkt", [D, CW], BF16, kind="ExternalInput").ap()
    wvt = nc.dram_tensor("wvt", [D, CW], BF16, kind="ExternalInput").ap()
    wot = nc.dram_tensor("wot", [CW, D], BF16, kind="ExternalInput").ap()
    bq = nc.dram_tensor("bq", [CW, 1], F32, kind="ExternalInput").ap()
    bk = nc.dram_tensor("bk", [CW, 1], F32, kind="ExternalInput").ap()
    bvb = nc.dram_tensor("bvb", [128, CW], F32, kind="ExternalInput").ap()
    out_p = nc.dram_tensor("out_p", [L, D], F32, kind="ExternalOutput").ap()

    with tile.TileContext(nc) as tc:
        with (
            tc.tile_pool(name="singles", bufs=1) as singles,
            tc.tile_pool(name="acts", bufs=1) as acts,
            tc.tile_pool(name="stage", bufs=12) as stage,
            tc.tile_pool(name="small", bufs=3) as small,
            tc.tile_pool(name="norm", bufs=5) as normp,
            tc.tile_pool(name="psq", bufs=2, space="PSUM") as ppq,
            tc.tile_pool(name="psa", bufs=2, space="PSUM") as ppa,
            tc.tile_pool(name="psc", bufs=2, space="PSUM") as ppc,
        ):
            ones = singles.tile([1, 64], F32)
            nc.vector.memset(ones, 1.0)

            w_sb = {}
            for name, dram in (("wq", wqt), ("wk", wkt), ("wv", wvt)):
                w = singles.tile([128, DC, CW], BF16, name=name + "_sb")
                nc.sync.dma_start(w, dram.rearrange("(c p) h -> p c h", p=128))
                w_sb[name] = w
            wot_sb = singles.tile([128, 2, D], BF16)
            nc.sync.dma_start(wot_sb, wot.rearrange("(g p) d -> p g d", p=128))
            b_sb = {}
            for name, dram in (("bq", bq), ("bk", bk)):
                b = singles.tile([128, 2], F32, name=name + "_sb")
                nc.sync.dma_start(b, dram.rearrange("(g p) one -> p (g one)", p=128))
                b_sb[name] = b
            bvb_sb = singles.tile([128, CW], F32)
            nc.sync.dma_start(bvb_sb, bvb)

            # all q/k/v chunk DMAs up-front: the DMA engines run far ahead
            # of compute, so every projection finds its data resident
            xcs = {}
            for ci in range(NCH):
                for name, dram in (("k", kT), ("v", vT), ("q", qT)):
                    xc = stage.tile(
                        [128, DC, 512], BF16, tag="xc", name=f"{name}c{ci}"
                    )
                    nc.sync.dma_start(
                        xc,
                        dram.rearrange("(c p) t -> p c t", p=128)[
                            :, :, ci * 512 : (ci + 1) * 512
                        ],
                    )
                    xcs[name, ci] = xc

            qm = acts.tile([128, 2, L], BF16)  # feature-major projections
            km = acts.tile([128, 2, L], BF16)
            vma = acts.tile([128, NT, HG * 66], BF16)  # [key, kt, (hv|1) x4]
            ctxn = acts.tile([128, 2, L], BF16)

            # ones columns of the augmented V (col 64 of each head's 66)
            nc.vector.memset(
                vma.rearrange("p t (h c) -> p t h c", c=66)[:, :, :, 64], 1.0
            )

            # warm the ACT exp table set during phase 1 instead of paying
            # the ~2.7us PSEUDO_LOAD at the first real exp
            dummy = singles.tile([128, 2], F32)
            nc.vector.memset(dummy[:, 0:1], 0.0)
            nc.scalar.activation(
                dummy[:, 1:2], dummy[:, 0:1], mybir.ActivationFunctionType.Exp
            )

            def fproj(name, ci):
                # feature-major projection of one 512-token chunk (k or q)
                dst, bias = (km, b_sb["bk"]) if name == "k" else (qm, b_sb["bq"])
                ts = slice(ci * 512, (ci + 1) * 512)
                for g in range(2):
                    ps = ppa.tile([128, 512], F32, tag="aux")
                    for dc in range(DC):
                        nc.tensor.matmul(
                            ps,
                            lhsT=w_sb["w" + name][:, dc, g * 128 : (g + 1) * 128],
                            rhs=xcs[name, ci][:, dc, :],
                            start=(dc == 0),
                            stop=(dc == DC - 1),
                        )
                    nc.vector.tensor_scalar_add(
                        dst[:, g, ts], ps, bias[:, g : g + 1]
                    )

            def vproj(ci):
                # token-major projection: vm[t, f], activations stationary
                for tt in range(4):
                    ps = ppa.tile([128, 512], F32, tag="aux")
                    for dc in range(DC):
                        nc.tensor.matmul(
                            ps[:, 0:CW],
                            lhsT=xcs["v", ci][:, dc, tt * 128 : (tt + 1) * 128],
                            rhs=w_sb["wv"][:, dc, :],
                            start=(dc == 0),
                            stop=(dc == DC - 1),
                        )
                    nc.vector.tensor_add(
                        vma.rearrange("p t (h c) -> p t h c", c=66)[
                            :, ci * 4 + tt, :, 0:64
                        ],
                        ps.rearrange("p (h c) -> p h c", c=64)[:, 0:HG, :],
                        bvb_sb.rearrange("p (h c) -> p h c", c=64),
                    )

            def emit_normalize(pg, pls, cus, prcs):
                for h in range(2):
                    po = ppa.tile([128, 512], F32, tag="aux")
                    nc.tensor.matmul(po[0:64, :], lhsT=ones, rhs=prcs[h])
                    nc.vector.tensor_mul(
                        ctxn[h * 64 : (h + 1) * 64, pg, pls],
                        cus[h][0:64, :],
                        po[0:64, :],
                    )

            def emit_outproj(lc):
                for tt in range(4):
                    t0 = lc * 512 + tt * 128
                    ob = small.tile([128, D], F32, tag="ob")
                    for half in range(2):
                        po = ppa.tile([128, 512], F32, tag="aux")
                        for g in range(2):
                            nc.tensor.matmul(
                                po,
                                lhsT=ctxn[:, g, t0 : t0 + 128],
                                rhs=wot_sb[:, g, half * 512 : (half + 1) * 512],
                                start=(g == 0),
                                stop=(g == 1),
                            )
                        nc.vector.tensor_copy(
                            ob[:, half * 512 : (half + 1) * 512], po
                        )
                    nc.sync.dma_start(out_p[t0 : t0 + 128, :], ob)

            # phase 1 head: just enough to start attention
            fproj("k", 0)
            vproj(0)
            fproj("q", 0)
            jobs = deque()
            for ci in range(1, NCH):
                jobs.append(lambda ci=ci: fproj("k", ci))
                jobs.append(lambda ci=ci: vproj(ci))

            # ---- phase 2: attention; deferred jobs drain at odd key tiles
            for lc in range(NCH):
                ls = slice(lc * 512, (lc + 1) * 512)
                for g in range(2):
                    ctx = [
                        ppc.tile([128, 512], F32, tag="ctx", name=f"ctx{h}")
                        for h in range(2)
                    ]
                    for pt in range(NT):
                        ks = slice(pt * 128, (pt + 1) * 128)
                        squad = ppq.tile([128, 2, 512], F32, tag="sq")
                        for h in range(2):
                            hs = slice(h * 64, (h + 1) * 64)
                            nc.tensor.matmul(
                                squad[:, h, :],
                                lhsT=km[hs, g, ks],
                                rhs=qm[hs, g, ls],
                                tile_position=(h * 64, 0),
                            )
                        es = small.tile([128, 2, 512], BF16, tag="es")
                        nc.scalar.activation(
                            es, squad, mybir.ActivationFunctionType.Exp, scale=SCALE
                        )
                        for h in range(2):
                            nc.tensor.matmul(
                                ctx[h][0:65, :],
                                lhsT=vma[
                                    :, pt, (g * 2 + h) * 66 : (g * 2 + h) * 66 + 65
                                ],
                                rhs=es[:, h, :],
                                start=(pt == 0),
                                stop=(pt == NT - 1),
                            )
                        if pt % 2 == 1 and jobs:
                            jobs.popleft()()
                    # evacuate ctx to SBUF (frees the PSUM banks) and start
                    # the reciprocal; the normalize runs as a later job
                    cus, rcs = [], []
                    for h in range(2):
                        cu = normp.tile([65, 512], F32, tag="cu")
                        nc.vector.tensor_copy(cu, ctx[h][0:65, :])
                        rc = normp.tile([1, 512], F32, tag="rc")
                        nc.vector.reciprocal(rc, cu[64:65, :])
                        cus.append(cu)
                        rcs.append(rc)
                    if g == 0 and lc < NCH - 1:
                        jobs.append(lambda ci=lc + 1: fproj("q", ci))
                    jobs.append(
                        lambda g=g, ls=ls, cus=cus, rcs=rcs: emit_normalize(
                            g, ls, cus, rcs
                        )
                    )
                    if g == 1:
                        jobs.append(lambda lc=lc: emit_outproj(lc))
            while jobs:
                jobs.popleft()()
    return _split_matmul_waits(nc)


_NC_CACHE = None


def kernel(q, k, v, attention_mask, Wq, bq, Wk, bk, Wv, bv, Wo, bo):
    global _NC_CACHE
    q, k, v = (np.asarray(x, np.float32) for x in (q, k, v))
    assert np.asarray(attention_mask).all(), "kernel assumes all-ones mask"
    if _NC_CACHE is None:
        _NC_CACHE = build_nc()
    nc = _NC_CACHE

    bfc = lambda x: np.ascontiguousarray(np.asarray(x, ml_dtypes.bfloat16))
    c = np.ascontiguousarray
    Wq, Wk, Wv, Wo = (np.asarray(x, np.float32) for x in (Wq, Wk, Wv, Wo))
    bq, bk, bv, bo = (np.asarray(x, np.float32) for x in (bq, bk, bv, bo))

    qT = [bfc(q[b].T) for b in range(B)]
    kT = [bfc(k[b].T) for b in range(B)]
    vT = [bfc(v[b].T) for b in range(B)]

    in_maps = []
    for ci in range(N_CORES):
        b = ci // (N_CORES // B)
        j = ci % (N_CORES // B)
        hs = slice(j * CW, (j + 1) * CW)
        in_maps.append(
            {
                "qT": qT[b],
                "kT": kT[b],
                "vT": vT[b],
                "wqt": bfc(Wq.T[:, hs]),
                "wkt": bfc(Wk.T[:, hs]),
                "wvt": bfc(Wv.T[:, hs]),
                "wot": bfc(Wo.T[hs, :]),
                "bq": c(bq[hs, None]),
                "bk": c(bk[hs, None]),
                "bvb": c(np.broadcast_to(bv[hs][None, :], (128, CW))),
            }
        )

    res = bass_utils.run_bass_kernel_spmd(
        nc,
        in_maps,
        core_ids=list(range(N_CORES)),
        tmpdir=os.environ.get("KERNEL_TMPDIR"),
    )
    globals()["LAST_RES"] = res
    out = np.zeros((B, L, D), np.float32)
    for ci, r in enumerate(res.results):
        out[ci // (N_CORES // B)] += r["out_p"]
    out += bo[None, None, :]
    return out
